# revision 18
# baseline (speedup 1.0000x reference)
"""Trainium2 Bass kernel for the attention-decoder step (nn_Decoder).

Computation (see reference):
  e      = emb_table[tok]                               [B, EMB]
  energy = tanh(h0 @ Wh.T + enc @ We.T + attn_b)        [B, T, H]
  scores = energy @ v ; probs = softmax_T(scores)       [B, T]
  ctx    = probs @ enc                                   [B, ENC]
  gates  = [e,ctx] @ W_ih.T + b_ih + h0 @ W_hh.T + b_hh  [B, 4H]
  c' = sig(f)*c0 + sig(i)*tanh(g); h' = sig(o)*tanh(c')
  pred   = [h',ctx,e] @ fc_W.T + fc_b                    [B, VSZ]

Sharding over 8 NeuronCores:
  - attention: data-parallel over B (8 rows/core; enc shard lives in SBUF)
  - LSTM: hidden-sharded (128 units/core, all 64 batch rows)
  - fc: vocab-sharded (4096 padded vocab rows/core)
  Two AllGathers (ctx and h_new^T, bf16) connect the phases.
All matmuls run in bf16 with fp32 PSUM accumulation; softmax/LSTM cell
elementwise math is fp32.
"""
import sys

if "/opt/trn_rl_repo" not in sys.path:
    sys.path.insert(0, "/opt/trn_rl_repo")

from contextlib import ExitStack

import numpy as np
import ml_dtypes

import concourse.bass as bass
import concourse.bacc as bacc
import concourse.tile as tile
from concourse import mybir
from concourse.bass_utils import run_bass_kernel_spmd
from concourse.masks import make_identity

BF16 = mybir.dt.bfloat16
F32 = mybir.dt.float32
AF = mybir.ActivationFunctionType

N_CORES = 8
B, T = 64, 512
VSZ, EMB, HID, ENC = 32000, 512, 1024, 1024
BSH = B // N_CORES            # 8 batch rows per core
HSH = HID // N_CORES          # 128 hidden units per core
VPAD = 32768
VSH = VPAD // N_CORES         # 4096 vocab rows per core
EC = ENC // 128               # 8 e-chunks
HT = HID // 128               # 8 h-tiles
NBT = BSH                     # 8 bt-chunks of 512 (one per local batch row)
GRP = 2                       # bt groups (PSUM: 4 energy + 4 scores banks)
GSZ = NBT // GRP              # 4 bt per group
XK = (EMB + ENC) // 128       # 12 x-side LSTM k-chunks
HK = HID // 128 + 1           # 9 h0-side chunks (incl. bias row)
HAUG = HK * 128               # 1152
# fc contraction k-chunk order: [ctx(8), e(4), bias+pad(1), h(8)]
FK = ENC // 128 + EMB // 128 + 1 + HID // 128    # 21
FAUG = FK * 128               # 2688
NVC = VSH // 512              # 8 vocab chunks of 512 per core

TRACE = False
LAST_EXEC_NS = None
STAGE = 5   # debug: 1=attn, 2=+AG1, 3=+LSTM, 4=+AG2, 5=full
# tensor_tensor_reduce crashes the exec unit on this runtime build
# (NRT_EXEC_UNIT_UNRECOVERABLE) — default to the two-op mult+reduce path.
TTR_MODE = 1  # 0=fused bf16 out, 1=two-op mult+reduce, 2=fused f32 out

_cache = {}


def _build(stage=None):
    stage = STAGE if stage is None else stage
    nc = bacc.Bacc("TRN2", target_bir_lowering=False, debug=False,
                   num_devices=N_CORES)

    # ---- I/O ----
    encT = nc.dram_tensor("encT", [ENC, BSH * T], BF16, kind="ExternalInput")
    WeT = nc.dram_tensor("WeT", [ENC, HID], BF16, kind="ExternalInput")
    WhT = nc.dram_tensor("WhT", [HID, HID], BF16, kind="ExternalInput")
    attn_bias = nc.dram_tensor("attn_bias", [HID, 1], F32, kind="ExternalInput")
    vT = nc.dram_tensor("vT", [HID, 1], BF16, kind="ExternalInput")
    h0a = nc.dram_tensor("h0a", [HID, BSH], BF16, kind="ExternalInput")
    h0aug = nc.dram_tensor("h0aug", [HAUG, B], BF16, kind="ExternalInput")
    c0sh = nc.dram_tensor("c0sh", [B, HSH], F32, kind="ExternalInput")
    eT = nc.dram_tensor("eT", [EMB, B], BF16, kind="ExternalInput")
    wihT = nc.dram_tensor("wihT", [(EMB + ENC), 4 * HSH], BF16,
                          kind="ExternalInput")
    whhT = nc.dram_tensor("whhT", [HAUG, 4 * HSH], BF16, kind="ExternalInput")
    fcwT = nc.dram_tensor("fcwT", [FAUG, VSH], BF16, kind="ExternalInput")

    pred_sh = nc.dram_tensor("pred_sh", [B, VSH], F32, kind="ExternalOutput")
    h_new_sh = nc.dram_tensor("h_new_sh", [B, HSH], F32, kind="ExternalOutput")
    c_new_sh = nc.dram_tensor("c_new_sh", [B, HSH], F32, kind="ExternalOutput")

    # ---- internal DRAM ----
    ag1_in = nc.dram_tensor("ag1_in", [ENC, BSH], BF16)
    ag1_out = nc.dram_tensor("ag1_out", [ENC * N_CORES, BSH], BF16,
                             addr_space="Shared")
    ag2_in = nc.dram_tensor("ag2_in", [HSH, B], BF16)
    ag2_out = nc.dram_tensor("ag2_out", [HSH * N_CORES, B], BF16,
                             addr_space="Shared")
    probs_dram = nc.dram_tensor("probs_dram", [NBT, T], BF16)
    scores_dram = nc.dram_tensor("scores_dram", [NBT, T], F32)
    rg = [list(range(N_CORES))]

    with tile.TileContext(nc) as tc, ExitStack() as ctx:
        const = ctx.enter_context(tc.tile_pool(name="const", bufs=1))

        # ---------- resident loads ----------
        WeT_t, encT_t, WhT_t, h0a_t = [], [], [], []
        for ec in range(EC):
            w = const.tile([128, HID], BF16, tag=f"WeT{ec}", name=f"WeT{ec}")
            nc.sync.dma_start(w[:], WeT[ec * 128:(ec + 1) * 128, :])
            WeT_t.append(w)
        for ec in range(EC):
            t = const.tile([128, BSH * T], BF16, tag=f"encT{ec}",
                           name=f"encTt{ec}")
            nc.sync.dma_start(t[:], encT[ec * 128:(ec + 1) * 128, :])
            encT_t.append(t)
        for ec in range(EC):
            w = const.tile([128, HID], BF16, tag=f"WhT{ec}", name=f"WhT{ec}")
            nc.sync.dma_start(w[:], WhT[ec * 128:(ec + 1) * 128, :])
            WhT_t.append(w)
            h = const.tile([128, BSH], BF16, tag=f"h0a{ec}", name=f"h0a{ec}")
            nc.sync.dma_start(h[:], h0a[ec * 128:(ec + 1) * 128, :])
            h0a_t.append(h)
        ab_t, vT_t = [], []
        for ht in range(HT):
            a = const.tile([128, 1], F32, tag=f"ab{ht}", name=f"ab{ht}")
            nc.sync.dma_start(a[:], attn_bias[ht * 128:(ht + 1) * 128, :])
            ab_t.append(a)
            v = const.tile([128, 1], BF16, tag=f"vT{ht}", name=f"vTt{ht}")
            nc.sync.dma_start(v[:], vT[ht * 128:(ht + 1) * 128, :])
            vT_t.append(v)

        xT_e = []
        for k in range(EMB // 128):
            t = const.tile([128, B], BF16, tag=f"xTe{k}", name=f"xTe{k}")
            nc.sync.dma_start(t[:], eT[k * 128:(k + 1) * 128, :])
            xT_e.append(t)
        h0aug_t = []
        for k in range(HK):
            t = const.tile([128, B], BF16, tag=f"h0aug{k}", name=f"h0augt{k}")
            nc.sync.dma_start(t[:], h0aug[k * 128:(k + 1) * 128, :])
            h0aug_t.append(t)
        wih_t = []
        for k in range(XK):
            t = const.tile([128, 4 * HSH], BF16, tag=f"wih{k}", name=f"wih{k}")
            nc.sync.dma_start(t[:], wihT[k * 128:(k + 1) * 128, :])
            wih_t.append(t)
        whh_t = []
        for k in range(HK):
            t = const.tile([128, 4 * HSH], BF16, tag=f"whh{k}", name=f"whh{k}")
            nc.sync.dma_start(t[:], whhT[k * 128:(k + 1) * 128, :])
            whh_t.append(t)
        c0_t = const.tile([B, HSH], F32, tag="c0", name="c0t")
        nc.sync.dma_start(c0_t[:], c0sh[:])

        identity_t = const.tile([128, 128], F32, tag="ident", name="identt")
        make_identity(nc, identity_t[:])

        ones_t = const.tile([128, B], BF16, tag="ones", name="onest")
        nc.vector.memset(ones_t[:], 0.0)
        nc.vector.memset(ones_t[0:1, :], 1.0)

        # softmax / ctx tiles (per bt-group, partition base 0)
        scores_t = [const.tile([GSZ, T], F32, tag=f"scores{g}",
                               name=f"scores{g}") for g in range(GRP)]
        nmax_t = [const.tile([GSZ, 1], F32, tag=f"nmax{g}", name=f"nmax{g}")
                  for g in range(GRP)]
        rsum_t = [const.tile([GSZ, 1], F32, tag=f"rsum{g}", name=f"rsum{g}")
                  for g in range(GRP)]
        rinv_t = [const.tile([GSZ, 1], F32, tag=f"rinv{g}", name=f"rinv{g}")
                  for g in range(GRP)]
        expd_t = [const.tile([GSZ, T], F32, tag=f"expd{g}", name=f"expd{g}")
                  for g in range(GRP)]
        probs_t = [const.tile([GSZ, T], BF16, tag=f"probs{g}",
                              name=f"probs{g}") for g in range(GRP)]
        probs_bc = [const.tile([128, T], BF16, tag=f"pbc{b}", name=f"pbc{b}")
                    for b in range(NBT)]
        ctxT_f = [const.tile([128, NBT], F32, tag=f"ctxf{ec}",
                             name=f"ctxf{ec}") for ec in range(EC)]
        ctxT_b = [const.tile([128, NBT], BF16, tag=f"ctxb{ec}",
                             name=f"ctxb{ec}") for ec in range(EC)]
        bias_bt = [const.tile([128, NBT], F32, tag=f"biasbt{ht}",
                              name=f"biasbt{ht}") for ht in range(HT)]

        # ---------- attention ----------
        with tc.tile_pool(name="mm_ps", bufs=GSZ, space="PSUM") as mm_ps, \
             tc.tile_pool(name="small_ps", bufs=4, space="PSUM") as small_ps, \
             tc.tile_pool(name="tanh_pool", bufs=3) as tanh_pool, \
             tc.tile_pool(name="scr_pool", bufs=2) as scr_pool:

            # P1: bias_bt[ht][:, b] = attn_b[ht] + (Wh @ h0_b)[ht]
            for ht in range(HT if stage >= 1.2 else 0):
                ps = small_ps.tile([128, BSH], F32, tag="smallps",
                                   name=f"whh0ps{ht}")
                for ec in range(EC):
                    nc.tensor.matmul(
                        ps[:], WhT_t[ec][:, ht * 128:(ht + 1) * 128],
                        h0a_t[ec][:], start=(ec == 0), stop=(ec == EC - 1))
                nc.scalar.activation(bias_bt[ht][:], ps[:], AF.Identity,
                                     bias=ab_t[ht][:])

            # P2: energy + tanh + scores, grouped by GSZ batch rows
            for grp in range(GRP if stage >= 1.3 else 0):
                lo, hi = grp * GSZ, (grp + 1) * GSZ
                sc_ps = [small_ps.tile([1, T], F32, tag="smallps",
                                       name=f"scps{grp}_{i}")
                         for i in range(GSZ)]
                for ht in range(HT):
                    e_ps = [mm_ps.tile([128, T], F32, tag="eps",
                                       name=f"eps{grp}_{ht}_{i}")
                            for i in range(GSZ)]
                    for ec in range(EC):
                        for bt in range(GSZ):
                            gb = lo + bt
                            nc.tensor.matmul(
                                e_ps[bt][:],
                                WeT_t[ec][:, ht * 128:(ht + 1) * 128],
                                encT_t[ec][:, gb * T:(gb + 1) * T],
                                start=(ec == 0), stop=(ec == EC - 1))
                    for bt in range(GSZ):
                        gb = lo + bt
                        th = tanh_pool.tile([128, T], BF16, tag="tanh",
                                            name=f"tanh{grp}_{ht}_{bt}")
                        nc.scalar.activation(
                            th[:], e_ps[bt][:], AF.Tanh,
                            bias=bias_bt[ht][:, gb:gb + 1])
                        nc.tensor.matmul(sc_ps[bt][:], vT_t[ht][:], th[:],
                                         start=(ht == 0), stop=(ht == HT - 1))
                for bt in range(GSZ):
                    stg = scr_pool.tile([1, T], F32, tag="scstg",
                                        name=f"scstg{grp}_{bt}")
                    nc.vector.tensor_copy(stg[:], sc_ps[bt][:])
                    nc.sync.dma_start(scores_dram[lo + bt:lo + bt + 1, :],
                                      stg[:])
                nc.sync.dma_start(scores_t[grp][:], scores_dram[lo:hi, :])

                if stage < 1.4:
                    continue
                # per-group softmax over T
                nc.vector.tensor_reduce(
                    nmax_t[grp][:], scores_t[grp][:],
                    axis=mybir.AxisListType.X, op=mybir.AluOpType.max,
                    negate=True)
                nc.scalar.activation(
                    expd_t[grp][:], scores_t[grp][:], AF.Exp,
                    bias=nmax_t[grp][:], accum_out=rsum_t[grp][:])
                nc.vector.reciprocal(rinv_t[grp][:], rsum_t[grp][:])
                nc.vector.tensor_scalar_mul(
                    probs_t[grp][:], expd_t[grp][:], rinv_t[grp][:])
                nc.sync.dma_start(probs_dram[lo:hi, :], probs_t[grp][:])

                if stage < 1.45:
                    continue
                # broadcast probs rows to 128 partitions; ctx via fused
                # multiply-reduce on DVE
                for bt in range(GSZ):
                    gb = lo + bt
                    pstg = scr_pool.tile([1, T], BF16, tag="pstg",
                                         name=f"pstg{gb}")
                    nc.sync.dma_start(pstg[:], probs_dram[gb:gb + 1, :])
                    nc.gpsimd.partition_broadcast(probs_bc[gb][:], pstg[:])
                    if stage < 1.5:
                        continue
                    for ec in range(EC):
                        if TTR_MODE == 3:
                            # mul only, no reduce, no ctxT write
                            scr = scr_pool.tile([128, T], BF16, tag="scr",
                                                name=f"scr{gb}_{ec}")
                            nc.vector.tensor_mul(
                                scr[:], encT_t[ec][:, gb * T:(gb + 1) * T],
                                probs_bc[gb][:])
                        elif TTR_MODE == 4:
                            # no probs_bc read; reduce into ctxT column
                            scr = scr_pool.tile([128, T], BF16, tag="scr",
                                                name=f"scr{gb}_{ec}")
                            nc.vector.tensor_mul(
                                scr[:], encT_t[ec][:, gb * T:(gb + 1) * T],
                                encT_t[ec][:, gb * T:(gb + 1) * T])
                            nc.vector.tensor_reduce(
                                ctxT_f[ec][:, gb:gb + 1], scr[:],
                                axis=mybir.AxisListType.X,
                                op=mybir.AluOpType.add)
                        elif TTR_MODE == 1:
                            scr = scr_pool.tile([128, T], BF16, tag="scr",
                                                name=f"scr{gb}_{ec}")
                            nc.vector.tensor_mul(
                                scr[:], encT_t[ec][:, gb * T:(gb + 1) * T],
                                probs_bc[gb][:])
                            nc.vector.tensor_reduce(
                                ctxT_f[ec][:, gb:gb + 1], scr[:],
                                axis=mybir.AxisListType.X,
                                op=mybir.AluOpType.add)
                        else:
                            dt = BF16 if TTR_MODE == 0 else F32
                            scr = scr_pool.tile([128, T], dt, tag="scr",
                                                name=f"scr{gb}_{ec}")
                            nc.vector.tensor_tensor_reduce(
                                out=scr[:],
                                in0=encT_t[ec][:, gb * T:(gb + 1) * T],
                                in1=probs_bc[gb][:],
                                scale=1.0, scalar=0.0,
                                op0=mybir.AluOpType.mult,
                                op1=mybir.AluOpType.add,
                                accum_out=ctxT_f[ec][:, gb:gb + 1])

        if stage >= 2:
            # ---------- AllGather ctx ----------
            for ec in range(EC):
                nc.scalar.copy(ctxT_b[ec][:], ctxT_f[ec][:])
                nc.sync.dma_start(ag1_in[ec * 128:(ec + 1) * 128, :],
                                  ctxT_b[ec][:])
            nc.gpsimd.collective_compute(
                "AllGather", mybir.AluOpType.bypass, replica_groups=rg,
                ins=[ag1_in[:]], outs=[ag1_out[:]])

            # reassemble ctx^T [ENC, B] tiles from per-core blocks
            xT_ctx = [const.tile([128, B], BF16, tag=f"xTc{ec}",
                                 name=f"xTc{ec}") for ec in range(EC)]
            ag1_view = ag1_out[:].rearrange("(r c p) b -> c p r b",
                                            r=N_CORES, c=EC, p=128)
            for ec in range(EC):
                nc.gpsimd.dma_start(
                    xT_ctx[ec][:].rearrange("p (r b) -> p r b", r=N_CORES),
                    ag1_view[ec])

        if stage >= 3:
            # ---------- LSTM (hidden-sharded, all 64 batch rows) ----------
            with tc.tile_pool(name="lstm_ps", bufs=2, space="PSUM") as lstm_ps:
                g_ps = lstm_ps.tile([B, 4 * HSH], F32, tag="lstmps",
                                    name="gps")
                chain = ([(xT_e[k], wih_t[k]) for k in range(EMB // 128)]
                         + [(xT_ctx[k], wih_t[EMB // 128 + k])
                            for k in range(ENC // 128)]
                         + [(h0aug_t[k], whh_t[k]) for k in range(HK)])
                for k, (lt, rt) in enumerate(chain):
                    nc.tensor.matmul(g_ps[:], lt[:], rt[:], start=(k == 0),
                                     stop=(k == len(chain) - 1))

                sig_if = const.tile([B, 2 * HSH], F32, tag="sigif",
                                    name="sigif")
                g_t = const.tile([B, HSH], F32, tag="gt", name="gt")
                o_t = const.tile([B, HSH], F32, tag="ot", name="ot")
                nc.scalar.activation(sig_if[:], g_ps[:, 0:2 * HSH], AF.Sigmoid)
                nc.scalar.activation(g_t[:], g_ps[:, 2 * HSH:3 * HSH], AF.Tanh)
                nc.scalar.activation(o_t[:], g_ps[:, 3 * HSH:4 * HSH],
                                     AF.Sigmoid)
                t1 = const.tile([B, HSH], F32, tag="t1", name="t1")
                t2 = const.tile([B, HSH], F32, tag="t2", name="t2")
                c_new = const.tile([B, HSH], F32, tag="cnew", name="cnew")
                tanh_c = const.tile([B, HSH], F32, tag="tanhc", name="tanhc")
                h_new = const.tile([B, HSH], F32, tag="hnew", name="hnew")
                nc.vector.tensor_mul(t1[:], sig_if[:, HSH:2 * HSH], c0_t[:])
                nc.vector.tensor_mul(t2[:], sig_if[:, 0:HSH], g_t[:])
                nc.vector.tensor_add(c_new[:], t1[:], t2[:])
                nc.scalar.activation(tanh_c[:], c_new[:], AF.Tanh)
                nc.vector.tensor_mul(h_new[:], o_t[:], tanh_c[:])
                nc.sync.dma_start(c_new_sh[:], c_new[:])
                nc.sync.dma_start(h_new_sh[:], h_new[:])

                # h_new^T for fc: PE transpose [64,128] -> [128,64]
                tr_ps = lstm_ps.tile([HSH, B], F32, tag="lstmps", name="trps")
                nc.tensor.transpose(tr_ps[:], h_new[:], identity_t[0:B, 0:B])
                h_newT_b = const.tile([HSH, B], BF16, tag="hnewTb",
                                      name="hnewTb")
                nc.scalar.copy(h_newT_b[:], tr_ps[:])
                nc.sync.dma_start(ag2_in[:], h_newT_b[:])

        if stage >= 4:
            nc.gpsimd.collective_compute(
                "AllGather", mybir.AluOpType.bypass, replica_groups=rg,
                ins=[ag2_in[:]], outs=[ag2_out[:]])
            featT_h = [const.tile([128, B], BF16, tag=f"fth{k}",
                                  name=f"fth{k}") for k in range(HID // 128)]
            for k in range(HID // 128):
                nc.sync.dma_start(featT_h[k][:],
                                  ag2_out[k * 128:(k + 1) * 128, :])

        if stage >= 5:
            # ---------- fc (vocab shard) ----------
            feat_list = xT_ctx + xT_e + [ones_t] + featT_h
            assert len(feat_list) == FK
            with tc.tile_pool(name="pred_ps", bufs=NVC, space="PSUM") as pps, \
                 tc.tile_pool(name="fcw_pool", bufs=5) as fcw_pool, \
                 tc.tile_pool(name="pout_pool", bufs=3) as pout_pool:
                pred_ps = [pps.tile([B, 512], F32, tag="predps",
                                    name=f"predps{i}") for i in range(NVC)]
                for f in range(FK):
                    fcw = fcw_pool.tile([128, VSH], BF16, tag="fcw",
                                        name=f"fcw{f}")
                    nc.sync.dma_start(fcw[:], fcwT[f * 128:(f + 1) * 128, :])
                    for vc in range(NVC):
                        nc.tensor.matmul(pred_ps[vc][:], feat_list[f][:],
                                         fcw[:, vc * 512:(vc + 1) * 512],
                                         start=(f == 0), stop=(f == FK - 1))
                for vc in range(NVC):
                    po = pout_pool.tile([B, 512], F32, tag="pout",
                                        name=f"pout{vc}")
                    nc.scalar.copy(po[:], pred_ps[vc][:])
                    nc.sync.dma_start(pred_sh[:, vc * 512:(vc + 1) * 512],
                                      po[:])

    nc.compile()
    return nc


def _bf16(a):
    return np.ascontiguousarray(a.astype(ml_dtypes.bfloat16))


def _f32(a):
    return np.ascontiguousarray(a, dtype=np.float32)


def _prep_inputs(tok, h, c, enc_out, emb_table, attn_W, attn_b, v_W,
                 W_ih, W_hh, b_ih, b_hh, fc_W, fc_b):
    tok = np.asarray(tok)
    h0 = np.asarray(h, dtype=np.float32)[0]          # [B, HID]
    c0 = np.asarray(c, dtype=np.float32)[0]          # [B, HID]
    enc = np.asarray(enc_out, dtype=np.float32)      # [B, T, ENC]
    emb = np.asarray(emb_table, dtype=np.float32)
    attn_W = np.asarray(attn_W, dtype=np.float32)    # [HID, ENC+HID]
    attn_b = np.asarray(attn_b, dtype=np.float32)
    v_W = np.asarray(v_W, dtype=np.float32)          # [1, HID]
    W_ih = np.asarray(W_ih, dtype=np.float32)        # [4H, EMB+ENC]
    W_hh = np.asarray(W_hh, dtype=np.float32)        # [4H, HID]
    b_g = np.asarray(b_ih, dtype=np.float32) + np.asarray(b_hh, np.float32)
    fc_W = np.asarray(fc_W, dtype=np.float32)        # [VSZ, HID+ENC+EMB]
    fc_b = np.asarray(fc_b, dtype=np.float32)

    e = emb[tok]                                     # [B, EMB] host gather
    eT16 = _bf16(e.T)

    WhT16 = _bf16(attn_W[:, :HID].T)
    WeT16 = _bf16(attn_W[:, HID:].T)
    ab = _f32(attn_b.reshape(HID, 1))
    vT16 = _bf16(v_W.reshape(1, HID).T)

    h0aug = np.zeros((HAUG, B), np.float32)
    h0aug[:HID] = h0.T
    h0aug[HID] = 1.0
    h0aug16 = _bf16(h0aug)

    # fc_WT_aug rows: [ctx(1024), e(512), bias+pad(128), h(1024)];
    # fc_W feature columns are [h, ctx, e]
    fcw_pad = np.zeros((VPAD, FAUG), np.float32)
    fcw_pad[:VSZ, 0:ENC] = fc_W[:, HID:HID + ENC]            # ctx block
    fcw_pad[:VSZ, ENC:ENC + EMB] = fc_W[:, HID + ENC:]       # e block
    fcw_pad[:VSZ, ENC + EMB] = fc_b                          # bias row
    fcw_pad[:VSZ, ENC + EMB + 128:] = fc_W[:, :HID]          # h block
    fcwT16 = _bf16(fcw_pad.T)                                # [FAUG, VPAD]

    in_maps = []
    for i in range(N_CORES):
        bs = slice(i * BSH, (i + 1) * BSH)
        hs = slice(i * HSH, (i + 1) * HSH)
        sel = np.concatenate([np.arange(g * HID + i * HSH,
                                        g * HID + (i + 1) * HSH)
                              for g in range(4)])
        wihT16 = _bf16(W_ih[sel].T)
        whh_aug = np.zeros((HAUG, 4 * HSH), np.float32)
        whh_aug[:HID] = W_hh[sel].T
        whh_aug[HID] = b_g[sel]
        whhT16 = _bf16(whh_aug)

        encT16 = _bf16(enc[bs].transpose(2, 0, 1).reshape(ENC, BSH * T))

        in_maps.append({
            "encT": encT16,
            "WeT": WeT16,
            "WhT": WhT16,
            "attn_bias": ab,
            "vT": vT16,
            "h0a": _bf16(h0.T[:, bs]),
            "h0aug": h0aug16,
            "c0sh": _f32(c0[:, hs]),
            "eT": eT16,
            "wihT": wihT16,
            "whhT": whhT16,
            "fcwT": np.ascontiguousarray(fcwT16[:, i * VSH:(i + 1) * VSH]),
        })
    return in_maps


def kernel(**inputs):
    global LAST_EXEC_NS
    key = ("nc", STAGE)
    if key not in _cache:
        _cache[key] = _build()
    nc = _cache[key]
    in_maps = _prep_inputs(**inputs)
    res = run_bass_kernel_spmd(nc, in_maps, list(range(N_CORES)),
                               trace=TRACE)
    LAST_EXEC_NS = res.exec_time_ns
    r = res.results
    pred = np.concatenate([r[i]["pred_sh"] for i in range(N_CORES)],
                          axis=1)[:, :VSZ]
    h_new = np.concatenate([r[i]["h_new_sh"] for i in range(N_CORES)], axis=1)
    c_new = np.concatenate([r[i]["c_new_sh"] for i in range(N_CORES)], axis=1)
    return (np.ascontiguousarray(pred, dtype=np.float32),
            h_new[None].astype(np.float32),
            c_new[None].astype(np.float32))


# revision 19
# speedup vs baseline: 1.0228x; 1.0228x over previous
"""Trainium2 Bass kernel for the attention-decoder step (nn_Decoder).

Computation (see reference):
  e      = emb_table[tok]                               [B, EMB]
  energy = tanh(h0 @ Wh.T + enc @ We.T + attn_b)        [B, T, H]
  scores = energy @ v ; probs = softmax_T(scores)       [B, T]
  ctx    = probs @ enc                                   [B, ENC]
  gates  = [e,ctx] @ W_ih.T + b_ih + h0 @ W_hh.T + b_hh  [B, 4H]
  c' = sig(f)*c0 + sig(i)*tanh(g); h' = sig(o)*tanh(c')
  pred   = [h',ctx,e] @ fc_W.T + fc_b                    [B, VSZ]

Sharding over 8 NeuronCores:
  - attention: data-parallel over B (8 rows/core; enc shard lives in SBUF)
  - LSTM: hidden-sharded (128 units/core, all 64 batch rows)
  - fc: vocab-sharded (4096 padded vocab rows/core)
  Two AllGathers (ctx and h_new^T, bf16) connect the phases.
All matmuls run in bf16 with fp32 PSUM accumulation; softmax/LSTM cell
elementwise math is fp32.

Scheduling notes (v2):
  - attention weights are DMA'd before the big enc shard so the Wh@h0
    prologue and first energy matmuls start early
  - scores matmuls for h-tile k are emitted after energy for h-tile k+1
    so the PE never stalls waiting for the tanh eviction
  - softmax/ctx run per 2-row group on DVE, overlapped with the next
    group's energy matmuls
  - LSTM gates start with the h0-side partial products (no AG1 dep);
    h_new is transposed with DVE block transposes (no PSUM), so the fc
    accumulators (8 PSUM banks) can open right after the gates bank frees
  - fc k-chunk order is [e, bias, ctx, h]: e/bias need no collective,
    ctx needs AG1, h needs AG2 — the AG2 latency hides under ctx matmuls
"""
import sys

if "/opt/trn_rl_repo" not in sys.path:
    sys.path.insert(0, "/opt/trn_rl_repo")

from contextlib import ExitStack

import numpy as np
import ml_dtypes

import concourse.bass as bass
import concourse.bacc as bacc
import concourse.tile as tile
from concourse import mybir
from concourse.bass_utils import run_bass_kernel_spmd

BF16 = mybir.dt.bfloat16
F32 = mybir.dt.float32
AF = mybir.ActivationFunctionType

N_CORES = 8
B, T = 64, 512
VSZ, EMB, HID, ENC = 32000, 512, 1024, 1024
BSH = B // N_CORES            # 8 batch rows per core
HSH = HID // N_CORES          # 128 hidden units per core
VPAD = 32768
VSH = VPAD // N_CORES         # 4096 vocab rows per core
EC = ENC // 128               # 8 e-chunks
HT = HID // 128               # 8 h-tiles
NBT = BSH                     # 8 bt-chunks of 512 (one per local batch row)
GRP = 4                       # bt groups (PSUM: 2 energy + 4 small banks)
GSZ = NBT // GRP              # 2 bt per group
XK = (EMB + ENC) // 128       # 12 x-side LSTM k-chunks
HK = HID // 128 + 1           # 9 h0-side chunks (incl. bias row)
HAUG = HK * 128               # 1152
# fc contraction k-chunk order: [e(4), bias+pad(1), ctx(8), h(8)]
FK = EMB // 128 + 1 + ENC // 128 + HID // 128    # 21
FAUG = FK * 128               # 2688
NVC = VSH // 512              # 8 vocab chunks of 512 per core

TRACE = False
LAST_EXEC_NS = None
STAGE = 5   # debug: 1=attn, 2=+AG1, 3=+LSTM, 4=+AG2, 5=full

_cache = {}


def _build(stage=None):
    stage = STAGE if stage is None else stage
    nc = bacc.Bacc("TRN2", target_bir_lowering=False, debug=False,
                   num_devices=N_CORES)

    # ---- I/O ----
    encT = nc.dram_tensor("encT", [ENC, BSH * T], BF16, kind="ExternalInput")
    WeT = nc.dram_tensor("WeT", [ENC, HID], BF16, kind="ExternalInput")
    WhT = nc.dram_tensor("WhT", [HID, HID], BF16, kind="ExternalInput")
    attn_bias = nc.dram_tensor("attn_bias", [HID, 1], F32,
                               kind="ExternalInput")
    vT = nc.dram_tensor("vT", [HID, 1], BF16, kind="ExternalInput")
    h0a = nc.dram_tensor("h0a", [HID, BSH], BF16, kind="ExternalInput")
    h0aug = nc.dram_tensor("h0aug", [HAUG, B], BF16, kind="ExternalInput")
    c0sh = nc.dram_tensor("c0sh", [B, HSH], F32, kind="ExternalInput")
    eT = nc.dram_tensor("eT", [EMB, B], BF16, kind="ExternalInput")
    wihT = nc.dram_tensor("wihT", [(EMB + ENC), 4 * HSH], BF16,
                          kind="ExternalInput")
    whhT = nc.dram_tensor("whhT", [HAUG, 4 * HSH], BF16, kind="ExternalInput")
    fcwT = nc.dram_tensor("fcwT", [FAUG, VSH], BF16, kind="ExternalInput")

    pred_sh = nc.dram_tensor("pred_sh", [B, VSH], F32, kind="ExternalOutput")
    h_new_sh = nc.dram_tensor("h_new_sh", [B, HSH], F32,
                              kind="ExternalOutput")
    c_new_sh = nc.dram_tensor("c_new_sh", [B, HSH], F32,
                              kind="ExternalOutput")

    # ---- internal DRAM ----
    ag1_in = nc.dram_tensor("ag1_in", [ENC, BSH], BF16)
    ag1_out = nc.dram_tensor("ag1_out", [ENC * N_CORES, BSH], BF16,
                             addr_space="Shared")
    ag2_in = nc.dram_tensor("ag2_in", [HSH, B], BF16)
    ag2_out = nc.dram_tensor("ag2_out", [HSH * N_CORES, B], BF16,
                             addr_space="Shared")
    probs_dram = nc.dram_tensor("probs_dram", [NBT, T], BF16)
    scores_dram = nc.dram_tensor("scores_dram", [NBT, T], F32)
    rg = [list(range(N_CORES))]

    with tile.TileContext(nc) as tc, ExitStack() as ctx:
        const = ctx.enter_context(tc.tile_pool(name="const", bufs=1))

        # ---------- resident loads (attention-critical first) ----------
        WhT_t, h0a_t = [], []
        for ec in range(EC):
            w = const.tile([128, HID], BF16, tag=f"WhT{ec}", name=f"WhT{ec}")
            nc.sync.dma_start(w[:], WhT[ec * 128:(ec + 1) * 128, :])
            WhT_t.append(w)
            h = const.tile([128, BSH], BF16, tag=f"h0a{ec}", name=f"h0a{ec}")
            nc.sync.dma_start(h[:], h0a[ec * 128:(ec + 1) * 128, :])
            h0a_t.append(h)
        ab_t, vT_t = [], []
        for ht in range(HT):
            a = const.tile([128, 1], F32, tag=f"ab{ht}", name=f"ab{ht}")
            nc.sync.dma_start(a[:], attn_bias[ht * 128:(ht + 1) * 128, :])
            ab_t.append(a)
            v = const.tile([128, 1], BF16, tag=f"vT{ht}", name=f"vTt{ht}")
            nc.sync.dma_start(v[:], vT[ht * 128:(ht + 1) * 128, :])
            vT_t.append(v)
        WeT_t, encT_t = [], []
        for ec in range(EC):
            w = const.tile([128, HID], BF16, tag=f"WeT{ec}", name=f"WeT{ec}")
            nc.sync.dma_start(w[:], WeT[ec * 128:(ec + 1) * 128, :])
            WeT_t.append(w)
        for ec in range(EC):
            t = const.tile([128, BSH * T], BF16, tag=f"encT{ec}",
                           name=f"encTt{ec}")
            nc.sync.dma_start(t[:], encT[ec * 128:(ec + 1) * 128, :])
            encT_t.append(t)

        xT_e = []
        for k in range(EMB // 128):
            t = const.tile([128, B], BF16, tag=f"xTe{k}", name=f"xTe{k}")
            nc.sync.dma_start(t[:], eT[k * 128:(k + 1) * 128, :])
            xT_e.append(t)
        h0aug_t = []
        for k in range(HK):
            t = const.tile([128, B], BF16, tag=f"h0aug{k}", name=f"h0augt{k}")
            nc.sync.dma_start(t[:], h0aug[k * 128:(k + 1) * 128, :])
            h0aug_t.append(t)
        wih_t = []
        for k in range(XK):
            t = const.tile([128, 4 * HSH], BF16, tag=f"wih{k}",
                           name=f"wih{k}")
            nc.sync.dma_start(t[:], wihT[k * 128:(k + 1) * 128, :])
            wih_t.append(t)
        whh_t = []
        for k in range(HK):
            t = const.tile([128, 4 * HSH], BF16, tag=f"whh{k}",
                           name=f"whh{k}")
            nc.sync.dma_start(t[:], whhT[k * 128:(k + 1) * 128, :])
            whh_t.append(t)
        c0_t = const.tile([B, HSH], F32, tag="c0", name="c0t")
        nc.sync.dma_start(c0_t[:], c0sh[:])

        ones_t = const.tile([128, B], BF16, tag="ones", name="onest")
        nc.vector.memset(ones_t[:], 0.0)
        nc.vector.memset(ones_t[0:1, :], 1.0)

        # softmax / ctx tiles (per bt-group, partition base 0)
        scores_t = [const.tile([GSZ, T], F32, tag=f"scores{g}",
                               name=f"scores{g}") for g in range(GRP)]
        nmax_t = [const.tile([GSZ, 1], F32, tag=f"nmax{g}", name=f"nmax{g}")
                  for g in range(GRP)]
        rsum_t = [const.tile([GSZ, 1], F32, tag=f"rsum{g}", name=f"rsum{g}")
                  for g in range(GRP)]
        rinv_t = [const.tile([GSZ, 1], F32, tag=f"rinv{g}", name=f"rinv{g}")
                  for g in range(GRP)]
        expd_t = [const.tile([GSZ, T], F32, tag=f"expd{g}", name=f"expd{g}")
                  for g in range(GRP)]
        probs_t = [const.tile([GSZ, T], BF16, tag=f"probs{g}",
                              name=f"probs{g}") for g in range(GRP)]
        probs_bc = [const.tile([128, T], BF16, tag=f"pbc{b}", name=f"pbc{b}")
                    for b in range(NBT)]
        ctxT_f = [const.tile([128, NBT], F32, tag=f"ctxf{ec}",
                             name=f"ctxf{ec}") for ec in range(EC)]
        ctxT_b = [const.tile([128, NBT], BF16, tag=f"ctxb{ec}",
                             name=f"ctxb{ec}") for ec in range(EC)]
        bias_bt = [const.tile([128, NBT], F32, tag=f"biasbt{ht}",
                              name=f"biasbt{ht}") for ht in range(HT)]

        # ---------- attention ----------
        with tc.tile_pool(name="mm_ps", bufs=2 * GSZ, space="PSUM") as mm_ps, \
             tc.tile_pool(name="small_ps", bufs=4, space="PSUM") as small_ps, \
             tc.tile_pool(name="tanh_pool", bufs=2 * GSZ + 2) as tanh_pool, \
             tc.tile_pool(name="scr_pool", bufs=3) as scr_pool:

            # P1: bias_bt[ht][:, b] = attn_b[ht] + (Wh @ h0_b)[ht]
            for ht in range(HT):
                ps = small_ps.tile([128, BSH], F32, tag="smallps",
                                   name=f"whh0ps{ht}")
                for ec in range(EC):
                    nc.tensor.matmul(
                        ps[:], WhT_t[ec][:, ht * 128:(ht + 1) * 128],
                        h0a_t[ec][:], start=(ec == 0), stop=(ec == EC - 1))
                nc.scalar.activation(bias_bt[ht][:], ps[:], AF.Identity,
                                     bias=ab_t[ht][:])

            # P2: energy + tanh + (lagged) scores, per bt-group
            for grp in range(GRP):
                lo, hi = grp * GSZ, (grp + 1) * GSZ
                sc_ps = [small_ps.tile([1, T], F32, tag="smallps",
                                       name=f"scps{grp}_{i}")
                         for i in range(GSZ)]
                tanh_tiles = {}
                for ht in range(HT + 1):
                    if ht < HT:
                        e_ps = [mm_ps.tile([128, T], F32, tag="eps",
                                           name=f"eps{grp}_{ht}_{i}")
                                for i in range(GSZ)]
                        for ec in range(EC):
                            for bt in range(GSZ):
                                gb = lo + bt
                                nc.tensor.matmul(
                                    e_ps[bt][:],
                                    WeT_t[ec][:, ht * 128:(ht + 1) * 128],
                                    encT_t[ec][:, gb * T:(gb + 1) * T],
                                    start=(ec == 0), stop=(ec == EC - 1))
                        for bt in range(GSZ):
                            gb = lo + bt
                            th = tanh_pool.tile(
                                [128, T], BF16, tag="tanh",
                                name=f"tanh{grp}_{ht}_{bt}")
                            nc.scalar.activation(
                                th[:], e_ps[bt][:], AF.Tanh,
                                bias=bias_bt[ht][:, gb:gb + 1])
                            tanh_tiles[(ht, bt)] = th
                    # scores for the previous h-tile (keeps PE off the
                    # ACT critical path)
                    if ht > 0:
                        for bt in range(GSZ):
                            nc.tensor.matmul(
                                sc_ps[bt][:], vT_t[ht - 1][:],
                                tanh_tiles.pop((ht - 1, bt))[:],
                                start=(ht - 1 == 0), stop=(ht - 1 == HT - 1))
                for bt in range(GSZ):
                    stg = scr_pool.tile([1, T], F32, tag="scstg",
                                        name=f"scstg{grp}_{bt}")
                    nc.vector.tensor_copy(stg[:], sc_ps[bt][:])
                    nc.sync.dma_start(scores_dram[lo + bt:lo + bt + 1, :],
                                      stg[:])
                nc.sync.dma_start(scores_t[grp][:], scores_dram[lo:hi, :])

                # per-group softmax over T
                nc.vector.tensor_reduce(
                    nmax_t[grp][:], scores_t[grp][:],
                    axis=mybir.AxisListType.X, op=mybir.AluOpType.max,
                    negate=True)
                nc.scalar.activation(
                    expd_t[grp][:], scores_t[grp][:], AF.Exp,
                    bias=nmax_t[grp][:], accum_out=rsum_t[grp][:])
                nc.vector.reciprocal(rinv_t[grp][:], rsum_t[grp][:])
                nc.vector.tensor_scalar_mul(
                    probs_t[grp][:], expd_t[grp][:], rinv_t[grp][:])
                nc.sync.dma_start(probs_dram[lo:hi, :], probs_t[grp][:])

                # probs row -> all 128 partitions (GpSimd), then ctx via
                # mult+reduce on DVE (tensor_tensor_reduce is broken on
                # this runtime build)
                for bt in range(GSZ):
                    gb = lo + bt
                    pstg = scr_pool.tile([1, T], BF16, tag="pstg",
                                         name=f"pstg{gb}")
                    nc.sync.dma_start(pstg[:], probs_dram[gb:gb + 1, :])
                    nc.gpsimd.partition_broadcast(probs_bc[gb][:], pstg[:])
                    for ec in range(EC):
                        scr = scr_pool.tile([128, T], BF16, tag="scr",
                                            name=f"scr{gb}_{ec}")
                        nc.vector.tensor_mul(
                            scr[:], encT_t[ec][:, gb * T:(gb + 1) * T],
                            probs_bc[gb][:])
                        nc.vector.tensor_reduce(
                            ctxT_f[ec][:, gb:gb + 1], scr[:],
                            axis=mybir.AxisListType.X,
                            op=mybir.AluOpType.add)

        if stage >= 2:
            # ---------- AllGather ctx ----------
            for ec in range(EC):
                nc.scalar.copy(ctxT_b[ec][:], ctxT_f[ec][:])
                nc.sync.dma_start(ag1_in[ec * 128:(ec + 1) * 128, :],
                                  ctxT_b[ec][:])
            nc.gpsimd.collective_compute(
                "AllGather", mybir.AluOpType.bypass, replica_groups=rg,
                ins=[ag1_in[:]], outs=[ag1_out[:]])

            # reassemble ctx^T [ENC, B] tiles from per-core blocks
            xT_ctx = [const.tile([128, B], BF16, tag=f"xTc{ec}",
                                 name=f"xTc{ec}") for ec in range(EC)]
            ag1_view = ag1_out[:].rearrange("(r c p) b -> c p r b",
                                            r=N_CORES, c=EC, p=128)
            for ec in range(EC):
                nc.gpsimd.dma_start(
                    xT_ctx[ec][:].rearrange("p (r b) -> p r b", r=N_CORES),
                    ag1_view[ec])

        if stage >= 3:
            # ---------- LSTM (hidden-sharded, all 64 batch rows) ----------
            with tc.tile_pool(name="lstm_ps", bufs=1,
                              space="PSUM") as lstm_ps:
                g_ps = lstm_ps.tile([B, 4 * HSH], F32, tag="lstmps",
                                    name="gps")
                # h0-side partials first: no AG1 dependency
                chain = ([(h0aug_t[k], whh_t[k]) for k in range(HK)]
                         + [(xT_e[k], wih_t[k]) for k in range(EMB // 128)]
                         + [(xT_ctx[k], wih_t[EMB // 128 + k])
                            for k in range(ENC // 128)])
                for k, (lt, rt) in enumerate(chain):
                    nc.tensor.matmul(g_ps[:], lt[:], rt[:], start=(k == 0),
                                     stop=(k == len(chain) - 1))

                sig_if = const.tile([B, 2 * HSH], F32, tag="sigif",
                                    name="sigif")
                g_t = const.tile([B, HSH], F32, tag="gt", name="gt")
                o_t = const.tile([B, HSH], F32, tag="ot", name="ot")
                nc.scalar.activation(sig_if[:], g_ps[:, 0:2 * HSH],
                                     AF.Sigmoid)
                nc.scalar.activation(g_t[:], g_ps[:, 2 * HSH:3 * HSH],
                                     AF.Tanh)
                nc.scalar.activation(o_t[:], g_ps[:, 3 * HSH:4 * HSH],
                                     AF.Sigmoid)
            t1 = const.tile([B, HSH], F32, tag="t1", name="t1")
            t2 = const.tile([B, HSH], F32, tag="t2", name="t2")
            c_new = const.tile([B, HSH], F32, tag="cnew", name="cnew")
            tanh_c = const.tile([B, HSH], F32, tag="tanhc", name="tanhc")
            h_new = const.tile([B, HSH], F32, tag="hnew", name="hnew")
            nc.vector.tensor_mul(t1[:], sig_if[:, HSH:2 * HSH], c0_t[:])
            nc.vector.tensor_mul(t2[:], sig_if[:, 0:HSH], g_t[:])
            nc.vector.tensor_add(c_new[:], t1[:], t2[:])
            nc.scalar.activation(tanh_c[:], c_new[:], AF.Tanh)
            nc.vector.tensor_mul(h_new[:], o_t[:], tanh_c[:])
            nc.sync.dma_start(c_new_sh[:], c_new[:])
            nc.sync.dma_start(h_new_sh[:], h_new[:])

            # h_new^T via DVE 32x32 block transposes (no PSUM use)
            h_newT = const.tile([HSH, B], F32, tag="hnewT", name="hnewT")
            for j in range(2):          # h_new partition blocks (64 rows)
                for i in range(4):      # h_new free blocks (128 cols)
                    nc.vector.transpose(
                        h_newT[i * 32:(i + 1) * 32, j * 32:(j + 1) * 32],
                        h_new[j * 32:(j + 1) * 32, i * 32:(i + 1) * 32])
            h_newT_b = const.tile([HSH, B], BF16, tag="hnewTb",
                                  name="hnewTb")
            nc.scalar.copy(h_newT_b[:], h_newT[:])
            nc.sync.dma_start(ag2_in[:], h_newT_b[:])

        if stage >= 4:
            nc.gpsimd.collective_compute(
                "AllGather", mybir.AluOpType.bypass, replica_groups=rg,
                ins=[ag2_in[:]], outs=[ag2_out[:]])
            featT_h = [const.tile([128, B], BF16, tag=f"fth{k}",
                                  name=f"fth{k}") for k in range(HID // 128)]
            for k in range(HID // 128):
                nc.sync.dma_start(featT_h[k][:],
                                  ag2_out[k * 128:(k + 1) * 128, :])

        if stage >= 5:
            # ---------- fc (vocab shard) ----------
            feat_list = xT_e + [ones_t] + xT_ctx + featT_h
            assert len(feat_list) == FK
            with tc.tile_pool(name="pred_ps", bufs=NVC, space="PSUM") as pps, \
                 tc.tile_pool(name="fcw_pool", bufs=5) as fcw_pool, \
                 tc.tile_pool(name="pout_pool", bufs=3) as pout_pool:
                pred_ps = [pps.tile([B, 512], F32, tag="predps",
                                    name=f"predps{i}") for i in range(NVC)]
                for f in range(FK):
                    fcw = fcw_pool.tile([128, VSH], BF16, tag="fcw",
                                        name=f"fcw{f}")
                    nc.sync.dma_start(fcw[:], fcwT[f * 128:(f + 1) * 128, :])
                    for vc in range(NVC):
                        nc.tensor.matmul(pred_ps[vc][:], feat_list[f][:],
                                         fcw[:, vc * 512:(vc + 1) * 512],
                                         start=(f == 0), stop=(f == FK - 1))
                for vc in range(NVC):
                    po = pout_pool.tile([B, 512], F32, tag="pout",
                                        name=f"pout{vc}")
                    nc.scalar.copy(po[:], pred_ps[vc][:])
                    nc.sync.dma_start(pred_sh[:, vc * 512:(vc + 1) * 512],
                                      po[:])

    nc.compile()
    return nc


def _bf16(a):
    return np.ascontiguousarray(a.astype(ml_dtypes.bfloat16))


def _f32(a):
    return np.ascontiguousarray(a, dtype=np.float32)


def _prep_inputs(tok, h, c, enc_out, emb_table, attn_W, attn_b, v_W,
                 W_ih, W_hh, b_ih, b_hh, fc_W, fc_b):
    tok = np.asarray(tok)
    h0 = np.asarray(h, dtype=np.float32)[0]          # [B, HID]
    c0 = np.asarray(c, dtype=np.float32)[0]          # [B, HID]
    enc = np.asarray(enc_out, dtype=np.float32)      # [B, T, ENC]
    emb = np.asarray(emb_table, dtype=np.float32)
    attn_W = np.asarray(attn_W, dtype=np.float32)    # [HID, ENC+HID]
    attn_b = np.asarray(attn_b, dtype=np.float32)
    v_W = np.asarray(v_W, dtype=np.float32)          # [1, HID]
    W_ih = np.asarray(W_ih, dtype=np.float32)        # [4H, EMB+ENC]
    W_hh = np.asarray(W_hh, dtype=np.float32)        # [4H, HID]
    b_g = np.asarray(b_ih, dtype=np.float32) + np.asarray(b_hh, np.float32)
    fc_W = np.asarray(fc_W, dtype=np.float32)        # [VSZ, HID+ENC+EMB]
    fc_b = np.asarray(fc_b, dtype=np.float32)

    e = emb[tok]                                     # [B, EMB] host gather
    eT16 = _bf16(e.T)

    WhT16 = _bf16(attn_W[:, :HID].T)
    WeT16 = _bf16(attn_W[:, HID:].T)
    ab = _f32(attn_b.reshape(HID, 1))
    vT16 = _bf16(v_W.reshape(1, HID).T)

    h0aug = np.zeros((HAUG, B), np.float32)
    h0aug[:HID] = h0.T
    h0aug[HID] = 1.0
    h0aug16 = _bf16(h0aug)

    # fc_WT_aug rows: [e(512), bias+pad(128), ctx(1024), h(1024)];
    # fc_W feature columns are [h, ctx, e]
    fcw_pad = np.zeros((VPAD, FAUG), np.float32)
    fcw_pad[:VSZ, 0:EMB] = fc_W[:, HID + ENC:]               # e block
    fcw_pad[:VSZ, EMB] = fc_b                                # bias row
    fcw_pad[:VSZ, EMB + 128:EMB + 128 + ENC] = fc_W[:, HID:HID + ENC]  # ctx
    fcw_pad[:VSZ, EMB + 128 + ENC:] = fc_W[:, :HID]          # h block
    fcwT16 = _bf16(fcw_pad.T)                                # [FAUG, VPAD]

    in_maps = []
    for i in range(N_CORES):
        bs = slice(i * BSH, (i + 1) * BSH)
        hs = slice(i * HSH, (i + 1) * HSH)
        sel = np.concatenate([np.arange(g * HID + i * HSH,
                                        g * HID + (i + 1) * HSH)
                              for g in range(4)])
        wihT16 = _bf16(W_ih[sel].T)
        whh_aug = np.zeros((HAUG, 4 * HSH), np.float32)
        whh_aug[:HID] = W_hh[sel].T
        whh_aug[HID] = b_g[sel]
        whhT16 = _bf16(whh_aug)

        encT16 = _bf16(enc[bs].transpose(2, 0, 1).reshape(ENC, BSH * T))

        in_maps.append({
            "encT": encT16,
            "WeT": WeT16,
            "WhT": WhT16,
            "attn_bias": ab,
            "vT": vT16,
            "h0a": _bf16(h0.T[:, bs]),
            "h0aug": h0aug16,
            "c0sh": _f32(c0[:, hs]),
            "eT": eT16,
            "wihT": wihT16,
            "whhT": whhT16,
            "fcwT": np.ascontiguousarray(fcwT16[:, i * VSH:(i + 1) * VSH]),
        })
    return in_maps


def kernel(**inputs):
    global LAST_EXEC_NS
    key = ("nc", STAGE)
    if key not in _cache:
        _cache[key] = _build()
    nc = _cache[key]
    in_maps = _prep_inputs(**inputs)
    res = run_bass_kernel_spmd(nc, in_maps, list(range(N_CORES)),
                               trace=TRACE)
    LAST_EXEC_NS = res.exec_time_ns
    r = res.results
    pred = np.concatenate([r[i]["pred_sh"] for i in range(N_CORES)],
                          axis=1)[:, :VSZ]
    h_new = np.concatenate([r[i]["h_new_sh"] for i in range(N_CORES)],
                           axis=1)
    c_new = np.concatenate([r[i]["c_new_sh"] for i in range(N_CORES)],
                           axis=1)
    return (np.ascontiguousarray(pred, dtype=np.float32),
            h_new[None].astype(np.float32),
            c_new[None].astype(np.float32))
